# revision 47
# baseline (speedup 1.0000x reference)
"""MultiHeadCrossAttention on 8 TRN2 NeuronCores.

Sharding: core c -> batch b = c//2, head-group g = c%2 (8 heads, 512 out dims).
Each core computes its head-group's Q/K/V projections, attention, and a
partial out-projection (Wo columns restricted to its head-group). Host sums
the two partials per batch and adds bo.

Softmax denominator trick: V is augmented with a ones-column per head
(via a zero weight column + bias 1.0), so attn@V_aug row 64 of each head's
65-wide block is the softmax denominator; normalize with DVE reciprocal +
a K=1 matmul broadcast, deferred one iteration so the reciprocal latency
hides under the next head's compute.

V_aug is packed 65 cols/head (520 total, not 128-padded): the V projection
streams only 520 output columns (PE cost is N-proportional), and PV runs
M=65 which costs the same as M=128.

PSUM evacuations alternate between DVE and ACT so phase-boundary drains
(all 8 banks at once) take half the serial time; ACT is otherwise idle
outside the exp phase.

All stream DMAs rotate through one shared pool with bufs == NUM_HWDGE_SEMS
(8) so slot reuse lands on the same DMA lane (WAW collapses into queue
order) and every DMA carries at most 2 semaphore waits (walrus limit).
"""

import sys

import numpy as np

if "/opt/trn_rl_repo" not in sys.path:
    sys.path.insert(0, "/opt/trn_rl_repo")

import concourse.bacc as bacc
import concourse.bass as bass
import concourse.mybir as mybir
import concourse.tile as tile
from concourse.bass_utils import run_bass_kernel_spmd

FP32 = mybir.dt.float32
FP32R = mybir.dt.float32r
FP16 = mybir.dt.float16

B, NQ, NK = 4, 1024, 2048
QD, KD = 1024, 768
H, D = 16, 64
E = H * D  # 1024 total embed dim
G = 8  # heads per core
GO = G * D  # 512 out dims per core
DA = D + 1  # 65: head dim + denominator column
GA = G * DA  # 520
HC = GA // 2  # 260: half of the packed V_aug width
GAP = 7 * DA + 128 + 8  # 591: V_aug SBUF width so every head has a 128-col LDW window
SCALE = 1.0 / 8.0

MMDT = FP16

# test.py hooks
TRACE = False
TRACE_KWARGS = {}
LAST_RESULT = None


def _mm(nc, out, lhsT, rhs, start, stop):
    nc.tensor.matmul(out, lhsT, rhs, start=start, stop=stop)


def build_program():
    nc = bacc.Bacc()

    qT = nc.declare_dram_parameter("qT", [QD, NQ], FP16, isOutput=False)
    kT = nc.declare_dram_parameter("kT", [KD, NK], FP16, isOutput=False)
    vT = nc.declare_dram_parameter("vT", [KD, NK], FP16, isOutput=False)
    wq = nc.declare_dram_parameter("wq", [QD, GO], FP16, isOutput=False)
    wk = nc.declare_dram_parameter("wk", [KD, GO], FP16, isOutput=False)
    wv = nc.declare_dram_parameter("wv", [KD, GA], FP16, isOutput=False)
    wo = nc.declare_dram_parameter("wo", [GO, E], MMDT, isOutput=False)
    vbias = nc.declare_dram_parameter("vbias", [128, GA], FP32, isOutput=False)
    bq = nc.declare_dram_parameter("bq", [128, 4], FP32, isOutput=False)
    bk = nc.declare_dram_parameter("bk", [128, 4], FP32, isOutput=False)
    out = nc.declare_dram_parameter("out", [NQ, E], FP32, isOutput=True)

    with (
        nc.allow_low_precision("fp16 attention activations; validated 1.7e-4 rel"),
        tile.TileContext(nc) as tc,
    ):
        with (
            tc.tile_pool(name="consts", bufs=1) as consts,
            tc.tile_pool(name="wo_p", bufs=1) as wo_p,
            tc.tile_pool(name="qt_p", bufs=1) as qt_p,
            tc.tile_pool(name="kt_p", bufs=1) as kt_p,
            tc.tile_pool(name="va_p", bufs=1) as va_p,
            tc.tile_pool(name="osb_p", bufs=1) as osb_p,
            tc.tile_pool(name="strm_p", bufs=1) as strm_p,
        ):
            bq_sb = consts.tile([128, 4], FP32)
            nc.sync.dma_start(bq_sb[:], bq[:, :])
            bk_sb = consts.tile([128, 4], FP32)
            nc.sync.dma_start(bk_sb[:], bk[:, :])
            ones_sb = consts.tile([1, 64], FP16)
            nc.vector.memset(ones_sb[:], 1.0)
            vbias_sb = consts.tile([128, GA], FP32)
            wo_sb = [wo_p.tile([128, E], MMDT, name=f"wo{kk}") for kk in range(4)]

            # Persistent activation tiles.
            # Qt[m][n]: [128 outdim, 512 tokq]   (m: outdim tile, n: tokq chunk)
            qt_sb = [
                [qt_p.tile([128, 512], FP16, name=f"qt{m}_{n}") for n in range(2)]
                for m in range(4)
            ]
            # Kt zero-padded per head-half: ktz[m][hl][c] is [128 outdim, 512 tokk]
            # where only partitions [hl*64, hl*64+64) hold data, the rest are 0.
            # Full-128-partition lhsT keeps the PE at 1 cyc/col (K=64 runs 2.5x
            # slower on real HW regardless of dtype).
            ktz = [
                [
                    [
                        kt_p.tile([128, 512], FP16, name=f"ktz{m}_{hl}_{c}")
                        for c in range(4)
                    ]
                    for hl in range(2)
                ]
                for m in range(4)
            ]
            for m in range(4):
                for c in range(4):
                    nc.vector.memset(ktz[m][0][c][64:128, :], 0.0)
                    nc.vector.memset(ktz[m][1][c][0:64, :], 0.0)
            # V_aug[t]: [128 tokk, 591]  (per-head 65-col blocks: 64 V dims,
            # col 64 = ones/denominator; cols 520+ are zeroed pad so head 7's
            # 128-col LDWEIGHTS window stays finite. PV loads a full 128-col
            # window per head — cols past 65 load the next head's data, which
            # only feeds ot partitions 65..127 that nobody reads — because a
            # 128-col stationary operand keeps FWL on (~25% faster matmuls).
            va_sb = [va_p.tile([128, GAP], FP16, name=f"va{t}") for t in range(16)]
            for t in range(16):
                nc.vector.memset(va_sb[t][:, GA:GAP], 0.0)
            # O^T (normalized) [concat dim 512 -> 4 tiles of 128, tokq 1024]
            osb = [osb_p.tile([128, NQ], MMDT, name=f"osb{t}") for t in range(4)]

            # fp16 stream buffers for qT/kT/vT.
            strm_tiles = [
                strm_p.tile([128, 512], FP16, name=f"strm{i}") for i in range(16)
            ]
            strm_ctr = [0]

            def strm():
                t = strm_tiles[strm_ctr[0] % 16]
                strm_ctr[0] += 1
                return t

            # ---- Phases B-D share the projection-weight scratch scope ----
            with (
                tc.tile_pool(name="wq_p", bufs=1) as wq_p,
                tc.tile_pool(name="wk_p", bufs=1) as wk_p,
                tc.tile_pool(name="wv_p", bufs=1) as wv_p,
            ):
                # DMA issue order: phase B's stream tiles interleaved with wq
                # first (each queue then carries only one ~128KB transfer
                # ahead of the first matmul; a queue moves ~20GB/s so every
                # weight tile queued ahead costs ~6us of PE start delay),
                # then wk/wv (trickle in during phase B), vbias/wo last
                # (needed at D-evac and F respectively).
                wq_sb = []
                qs_tiles = []
                for kk in range(8):
                    qs = strm()
                    nc.sync.dma_start(
                        qs[:], qT[kk * 128 : (kk + 1) * 128, 0:512]
                    )
                    t = wq_p.tile([128, GO], FP16, name=f"wq{kk}")
                    nc.sync.dma_start(t[:], wq[kk * 128 : (kk + 1) * 128, :])
                    wq_sb.append(t)
                    qs2 = strm()
                    nc.sync.dma_start(
                        qs2[:], qT[kk * 128 : (kk + 1) * 128, 512:1024]
                    )
                    qs_tiles.append((qs, qs2))
                wk_sb = []
                for kk in range(6):
                    t = wk_p.tile([128, GO], FP16, name=f"wk{kk}")
                    nc.sync.dma_start(t[:], wk[kk * 128 : (kk + 1) * 128, :])
                    wk_sb.append(t)
                wv_sb = []
                for kk in range(6):
                    t = wv_p.tile([128, GA], FP16, name=f"wv{kk}")
                    nc.sync.dma_start(t[:], wv[kk * 128 : (kk + 1) * 128, :])
                    wv_sb.append(t)

                # Phases B, C, D share ONE PSUM pool with 8 shared tile tags
                # (ps{i}_{j}, all [128,512] fp32, one bank each). A per-phase
                # pool scope would barrier the next phase's first matmul on
                # ALL eight evacuations + scope close (measured 5.3us at B->C
                # and 2.9us at C->D); shared tags instead make each bank's
                # next producer wait only that bank's own evacuation, which
                # lands while the other banks' matmuls still run.
                with tc.tile_pool(name="psP", bufs=1, space="PSUM") as psP:
                    # ---- Phase B: Q projection. Qt = Wq_g @ query^T (+bq) ----
                    psq = [
                        [
                            psP.tile([128, 512], FP32, name=f"ps{m}_{n}")
                            for n in range(2)
                        ]
                        for m in range(4)
                    ]
                    # n-outer so n=0's evacuation overlaps n=1's matmuls.
                    for n in range(2):
                        for kk in range(8):
                            qs = qs_tiles[kk][n]
                            for m in range(4):
                                _mm(
                                    nc,
                                    psq[m][n][:],
                                    wq_sb[kk][:, m * 128 : (m + 1) * 128],
                                    qs[:],
                                    start=(kk == 0),
                                    stop=(kk == 7),
                                )
                        for m in range(4):
                            if m % 2 == 0:
                                nc.vector.tensor_scalar_add(
                                    qt_sb[m][n][:], psq[m][n][:], bq_sb[:, m : m + 1]
                                )
                            else:
                                nc.scalar.activation(
                                    qt_sb[m][n][:],
                                    psq[m][n][:],
                                    mybir.ActivationFunctionType.Identity,
                                    bias=bq_sb[:, m : m + 1],
                                    scale=1.0,
                                )

                    # ---- Phase C: K projection. Kt = Wk_g @ key^T (+bk) ----
                    # (half, n)-outer: each 4-bank group's evacuation overlaps
                    # the next group's matmuls.
                    for half in range(2):
                        for n in range(2):
                            psk = [
                                psP.tile([128, 512], FP32, name=f"ps{m}_{n}")
                                for m in range(4)
                            ]
                            for kk in range(6):
                                ks_ = strm()
                                c0 = half * 1024 + n * 512
                                nc.sync.dma_start(
                                    ks_[:],
                                    kT[kk * 128 : (kk + 1) * 128, c0 : c0 + 512],
                                )
                                for m in range(4):
                                    _mm(
                                        nc,
                                        psk[m][:],
                                        wk_sb[kk][:, m * 128 : (m + 1) * 128],
                                        ks_[:],
                                        start=(kk == 0),
                                        stop=(kk == 5),
                                    )
                            for m in range(4):
                                c = half * 2 + n
                                nc.vector.tensor_scalar_add(
                                    ktz[m][0][c][0:64, :],
                                    psk[m][0:64, :],
                                    bk_sb[0:64, m : m + 1],
                                )
                                nc.scalar.activation(
                                    ktz[m][1][c][64:128, :],
                                    psk[m][64:128, :],
                                    mybir.ActivationFunctionType.Identity,
                                    bias=bk_sb[64:128, m : m + 1],
                                    scale=1.0,
                                )
                    # Heavy cold-phase loads issued after phase C's streams so
                    # no kT stream queues behind a 128KB wo half-tile: vbias
                    # is needed at D's first evacuation, wo only in phase F.
                    nc.sync.dma_start(vbias_sb[:, 0:HC], vbias[:, 0:HC])
                    nc.sync.dma_start(vbias_sb[:, HC:GA], vbias[:, HC:GA])
                    for kk in range(4):
                        for hv in range(2):
                            nc.sync.dma_start(
                                wo_sb[kk][:, hv * 512 : (hv + 1) * 512],
                                wo[
                                    kk * 128 : (kk + 1) * 128,
                                    hv * 512 : (hv + 1) * 512,
                                ],
                            )

                    # ---- Phase D: V_aug = value @ Wv_aug^T (+vbias) ----
                    # Packed 520-wide output (2 chunks of 260 per token tile).
                    for tb in range(4):
                        psv = [
                            [
                                psP.tile([128, 512], FP32, name=f"ps{t2}_{c}")
                                for c in range(2)
                            ]
                            for t2 in range(4)
                        ]
                        for kk in range(6):
                            vs = strm()
                            nc.sync.dma_start(
                                vs[:],
                                vT[kk * 128 : (kk + 1) * 128, tb * 512 : (tb + 1) * 512],
                            )
                            for t2 in range(4):
                                for c in range(2):
                                    _mm(
                                        nc,
                                        psv[t2][c][:, 0:HC],
                                        vs[:, t2 * 128 : (t2 + 1) * 128],
                                        wv_sb[kk][:, c * HC : (c + 1) * HC],
                                        start=(kk == 0),
                                        stop=(kk == 5),
                                    )
                        for t2 in range(4):
                            for c in range(2):
                                nc.vector.tensor_add(
                                    va_sb[tb * 4 + t2][:, c * HC : (c + 1) * HC],
                                    psv[t2][c][:, 0:HC],
                                    vbias_sb[:, c * HC : (c + 1) * HC],
                                )

            # ---- Phase E: attention per head (+ first half of out-proj) ----
            # Exp grouped into [128,1024] 2-bank ACT calls (halves per-call
            # overhead). Normalize chain (recip -> K=1 broadcast mm into the
            # pending ot's partitions 64..127 -> mul) is deferred by one
            # (h,n) iteration, and recip is enqueued on DVE BEFORE the flush
            # ops so the DVE FIFO can't hold it hostage behind a copy that
            # waits on the PE.
            # Iteration order is n-major: after the 8 (h, n=0) flushes the
            # osb columns 0..511 are final, so out-proj m-tiles 0..3 slot
            # into the PE slack of iterations 9..12 while ACT grinds exp.
            with (
                tc.tile_pool(name="otp", bufs=3, space="PSUM") as otp,
                tc.tile_pool(name="stp", bufs=2, space="PSUM") as stp,
                tc.tile_pool(name="psF", bufs=1, space="PSUM") as psF,
                tc.tile_pool(name="p_p", bufs=4) as p_p,
                tc.tile_pool(name="rc_p", bufs=2) as rc_p,
                tc.tile_pool(name="nr_p", bufs=2) as nr_p,
                tc.tile_pool(name="bcs_p", bufs=2) as bcs_p,
                tc.tile_pool(name="ys_p", bufs=2) as ys_p,
            ):
                iters = [(h, n) for n in range(2) for h in range(G)]
                # Two-deep normalize pipeline. A PE instruction's cross-engine
                # wait collapses onto the latest program-order-preceding
                # instruction of the depended-on engine, and that engine
                # completes in FIFO order — so the 3.4us single-partition DVE
                # InstReciprocal stalled every PE op with any DVE dependency
                # emitted in its shadow (~2us/iter, measured). Instead the
                # denominator row is reshaped onto 4 partitions by an
                # SBUF->SBUF DMA, the reciprocal runs partition-parallel as
                # [4,128] (~1.1us), a second DMA reshapes it back, and the
                # recip is emitted at pg==1 while the flush of the
                # two-iterations-ago head lands at pg==5 — so the only DVE
                # instructions next to PE emission points are sub-us copies.
                pending = [None, None]  # [one-ago, two-ago]
                stage1 = [None]  # entry awaiting its pg==1 reciprocal
                ot_cur = [None]

                def flush_oldest():
                    e = pending[1]
                    ot_p = e["ot"]
                    nc.tensor.matmul(
                        ot_p[64:128, :],
                        ones_sb[:, :],
                        e["rc"][:],
                        start=True,
                        stop=True,
                    )
                    bcs = bcs_p.tile([64, 512], FP16, name="bcs")
                    nc.vector.tensor_copy(bcs[:], ot_p[64:128, :])
                    nc.vector.tensor_mul(
                        osb[e["mt"]][
                            e["po"] : e["po"] + 64, e["n"] * 512 : (e["n"] + 1) * 512
                        ],
                        ot_p[:64, :],
                        bcs[:],
                    )
                    pending[1] = None

                def emit_recip():
                    e = stage1[0]
                    rc4 = nr_p.tile([4, 128], FP16, name="rc4")
                    nc.vector.reciprocal(rc4[:], e["den4"][:])
                    rc = rc_p.tile([1, 512], FP16, name="rc")
                    nc.sync.dma_start(rc[:], rc4[:])
                    e["rc"] = rc
                    stage1[0] = None

                def do_pv(pit, pg, pp):
                    ph, pn = iters[pit]
                    if pg == 0:
                        ot_cur[0] = otp.tile([128, 512], FP32, name="ot")
                    if pg == 1 and stage1[0] is not None:
                        emit_recip()
                    if pg == 5 and pending[1] is not None:
                        flush_oldest()
                    ot = ot_cur[0]
                    for j in range(2):
                        _mm(
                            nc,
                            ot[:],
                            va_sb[2 * pg + j][:, ph * DA : ph * DA + 128],
                            pp[:, j * 512 : (j + 1) * 512],
                            start=(pg == 0 and j == 0),
                            stop=(pg == 7 and j == 1),
                        )
                    if pg == 7:
                        # Denominator row off PSUM (sub-us DVE copy), then
                        # spread over 4 partitions for the cheap reciprocal.
                        den = nr_p.tile([1, 512], FP32, name="den")
                        nc.vector.tensor_copy(den[:], ot[64:65, :])
                        den4 = nr_p.tile([4, 128], FP32, name="den4")
                        nc.sync.dma_start(den4[:], den[:])
                        assert pending[1] is None
                        e = {
                            "ot": ot,
                            "den4": den4,
                            "rc": None,
                            "mt": ph // 2,
                            "po": (ph % 2) * 64,
                            "n": pn,
                        }
                        stage1[0] = e
                        pending[1] = pending[0]
                        pending[0] = e

                def do_F(m, psF_pool, ys_pool, dve_only):
                    ys = ys_pool.tile([128, 1024], FP32, name="ys")
                    for n2 in range(2):
                        psy = psF_pool.tile([128, 512], FP32, name="psy")
                        for kt in range(4):
                            _mm(
                                nc,
                                psy[:],
                                osb[kt][:, m * 128 : (m + 1) * 128],
                                wo_sb[kt][:, n2 * 512 : (n2 + 1) * 512],
                                start=(kt == 0),
                                stop=(kt == 3),
                            )
                        if dve_only or n2 == 0:
                            nc.vector.tensor_copy(
                                ys[:, n2 * 512 : (n2 + 1) * 512], psy[:]
                            )
                        else:
                            nc.scalar.copy(ys[:, 512:1024], psy[:])
                    for hv in range(2):
                        nc.sync.dma_start(
                            out[m * 128 : (m + 1) * 128, hv * 512 : (hv + 1) * 512],
                            ys[:, hv * 512 : (hv + 1) * 512],
                        )

                queue = []
                for it in range(16):
                    h, n = iters[it]
                    mt, hl = h // 2, h % 2
                    for g2 in range(8):
                        st2 = stp.tile([128, 1024], FP32, name="st")
                        for j in range(2):
                            kt = 2 * g2 + j
                            _mm(
                                nc,
                                st2[:, j * 512 : (j + 1) * 512],
                                ktz[mt][hl][kt // 4][
                                    :, (kt % 4) * 128 : (kt % 4 + 1) * 128
                                ],
                                qt_sb[mt][n][:],
                                start=True,
                                stop=True,
                            )
                        p2 = p_p.tile([128, 1024], FP16, name="p")
                        nc.scalar.activation(
                            p2[:],
                            st2[:],
                            mybir.ActivationFunctionType.Exp,
                            bias=0.0,
                            scale=SCALE,
                        )
                        queue.append((it, g2, p2))
                        if len(queue) >= 2:
                            do_pv(*queue.pop(0))
                    if 9 <= it <= 12:
                        # ACT is saturated with exp here; keep its queue clear.
                        do_F(it - 9, psF, ys_p, dve_only=True)
                do_pv(*queue.pop(0))
                if stage1[0] is not None:
                    emit_recip()
                if pending[1] is not None:
                    flush_oldest()
                pending[1] = pending[0]
                pending[0] = None
                flush_oldest()

            # ---- Phase F tail: out-proj m-tiles 4..7 (tokq 512..1023) ----
            with (
                tc.tile_pool(name="psF2", bufs=4, space="PSUM") as psF2,
                tc.tile_pool(name="ys2_p", bufs=2) as ys2_p,
            ):
                for m in range(4, 8):
                    do_F(m, psF2, ys2_p, dve_only=False)

    nc.finalize()
    return nc


def kernel(**inputs):
    global LAST_RESULT
    arrs = {k: np.asarray(v, dtype=np.float32) for k, v in inputs.items()}
    query, key, value = arrs["query"], arrs["key"], arrs["value"]
    Wq, bq_, Wk, bk_ = arrs["Wq"], arrs["bq"], arrs["Wk"], arrs["bk"]
    Wv, bv_, Wo, bo_ = arrs["Wv"], arrs["bv"], arrs["Wo"], arrs["bo"]

    nc = build_program()

    qTb = [np.ascontiguousarray(query[b].T.astype(np.float16)) for b in range(B)]
    kTb = [np.ascontiguousarray(key[b].T.astype(np.float16)) for b in range(B)]
    vTb = [np.ascontiguousarray(value[b].T.astype(np.float16)) for b in range(B)]

    per_group = []
    for g in range(2):
        gs = slice(g * GO, (g + 1) * GO)
        wq_m = np.ascontiguousarray(Wq[gs, :].T.astype(np.float16))
        wk_m = np.ascontiguousarray(Wk[gs, :].T.astype(np.float16))
        wv_aug = np.zeros((KD, GA), np.float32)
        vb_row = np.zeros((GA,), np.float32)
        for h in range(G):
            hs = slice(g * GO + h * D, g * GO + (h + 1) * D)
            wv_aug[:, h * DA : h * DA + D] = Wv[hs, :].T
            vb_row[h * DA : h * DA + D] = bv_[hs]
            vb_row[h * DA + D] = 1.0
        vbias_m = np.ascontiguousarray(np.tile(vb_row, (128, 1)).astype(np.float32))
        wo_m = np.ascontiguousarray(Wo[:, gs].T.astype(np.float16))
        bq_m = np.ascontiguousarray(bq_[gs].reshape(4, 128).T)
        bk_m = np.ascontiguousarray(bk_[gs].reshape(4, 128).T)
        per_group.append(
            {
                "wq": wq_m,
                "wk": wk_m,
                "wv": wv_aug.astype(np.float16),
                "wo": wo_m,
                "vbias": vbias_m,
                "bq": bq_m,
                "bk": bk_m,
            }
        )

    in_maps = []
    for c in range(8):
        b, g = c // 2, c % 2
        m = {"qT": qTb[b], "kT": kTb[b], "vT": vTb[b]}
        m.update(per_group[g])
        in_maps.append(m)

    res = run_bass_kernel_spmd(
        nc, in_maps, list(range(8)), trace=TRACE, **(TRACE_KWARGS if TRACE else {})
    )
    LAST_RESULT = res

    outs = res.results
    Y = np.empty((B, NQ, E), np.float32)
    for b in range(B):
        Y[b] = outs[2 * b]["out"] + outs[2 * b + 1]["out"] + bo_[None, :]
    return Y


# revision 49
# speedup vs baseline: 1.0132x; 1.0132x over previous
"""MultiHeadCrossAttention on 8 TRN2 NeuronCores.

Sharding: core c -> batch b = c//2, head-group g = c%2 (8 heads, 512 out dims).
Each core computes its head-group's Q/K/V projections, attention, and a
partial out-projection (Wo columns restricted to its head-group). Host sums
the two partials per batch and adds bo.

Softmax denominator trick: V is augmented with a ones-column per head
(via a zero weight column + bias 1.0), so attn@V_aug row 64 of each head's
65-wide block is the softmax denominator; normalize with DVE reciprocal +
a K=1 matmul broadcast, deferred one iteration so the reciprocal latency
hides under the next head's compute.

V_aug is packed 65 cols/head (520 total, not 128-padded): the V projection
streams only 520 output columns (PE cost is N-proportional), and PV runs
M=65 which costs the same as M=128.

PSUM evacuations alternate between DVE and ACT so phase-boundary drains
(all 8 banks at once) take half the serial time; ACT is otherwise idle
outside the exp phase.

All stream DMAs rotate through one shared pool with bufs == NUM_HWDGE_SEMS
(8) so slot reuse lands on the same DMA lane (WAW collapses into queue
order) and every DMA carries at most 2 semaphore waits (walrus limit).
"""

import sys

import numpy as np

if "/opt/trn_rl_repo" not in sys.path:
    sys.path.insert(0, "/opt/trn_rl_repo")

import concourse.bacc as bacc
import concourse.bass as bass
import concourse.mybir as mybir
import concourse.tile as tile
from concourse.bass_utils import run_bass_kernel_spmd

FP32 = mybir.dt.float32
FP32R = mybir.dt.float32r
FP16 = mybir.dt.float16

B, NQ, NK = 4, 1024, 2048
QD, KD = 1024, 768
H, D = 16, 64
E = H * D  # 1024 total embed dim
G = 8  # heads per core
GO = G * D  # 512 out dims per core
DA = D + 1  # 65: head dim + denominator column
GA = G * DA  # 520
HC = GA // 2  # 260: half of the packed V_aug width
GAP = 7 * DA + 128 + 8  # 591: V_aug SBUF width so every head has a 128-col LDW window
SCALE = 1.0 / 8.0

MMDT = FP16

# test.py hooks
TRACE = False
TRACE_KWARGS = {}
LAST_RESULT = None


def _mm(nc, out, lhsT, rhs, start, stop):
    nc.tensor.matmul(out, lhsT, rhs, start=start, stop=stop)


def build_program():
    nc = bacc.Bacc()

    qT = nc.declare_dram_parameter("qT", [QD, NQ], FP16, isOutput=False)
    kT = nc.declare_dram_parameter("kT", [KD, NK], FP16, isOutput=False)
    vT = nc.declare_dram_parameter("vT", [KD, NK], FP16, isOutput=False)
    wq = nc.declare_dram_parameter("wq", [QD, GO], FP16, isOutput=False)
    wk = nc.declare_dram_parameter("wk", [KD, GO], FP16, isOutput=False)
    wv = nc.declare_dram_parameter("wv", [KD, GA], FP16, isOutput=False)
    wo = nc.declare_dram_parameter("wo", [GO, E], MMDT, isOutput=False)
    vbias = nc.declare_dram_parameter("vbias", [128, GA], FP32, isOutput=False)
    bq = nc.declare_dram_parameter("bq", [128, 4], FP32, isOutput=False)
    bk = nc.declare_dram_parameter("bk", [128, 4], FP32, isOutput=False)
    # Unused; distinguishes this program revision in the NEFF cache key (the
    # cache was observed to false-hit across a pool-structure-only change).
    rev = nc.declare_dram_parameter("rev11", [1, 4], FP32, isOutput=False)
    out = nc.declare_dram_parameter("out", [NQ, E], FP32, isOutput=True)

    with (
        nc.allow_low_precision("fp16 attention activations; validated 1.7e-4 rel"),
        tile.TileContext(nc) as tc,
    ):
        with (
            tc.tile_pool(name="consts", bufs=1) as consts,
            tc.tile_pool(name="wo_p", bufs=1) as wo_p,
            tc.tile_pool(name="qt_p", bufs=1) as qt_p,
            tc.tile_pool(name="kt_p", bufs=1) as kt_p,
            tc.tile_pool(name="va_p", bufs=1) as va_p,
            tc.tile_pool(name="osb_p", bufs=1) as osb_p,
            tc.tile_pool(name="strm_p", bufs=1) as strm_p,
        ):
            bq_sb = consts.tile([128, 4], FP32)
            nc.sync.dma_start(bq_sb[:], bq[:, :])
            bk_sb = consts.tile([128, 4], FP32)
            nc.sync.dma_start(bk_sb[:], bk[:, :])
            ones_sb = consts.tile([1, 64], FP16)
            nc.vector.memset(ones_sb[:], 1.0)
            vbias_sb = consts.tile([128, GA], FP32)
            wo_sb = [wo_p.tile([128, E], MMDT, name=f"wo{kk}") for kk in range(4)]

            # Persistent activation tiles.
            # Qt[m][n]: [128 outdim, 512 tokq]   (m: outdim tile, n: tokq chunk)
            qt_sb = [
                [qt_p.tile([128, 512], FP16, name=f"qt{m}_{n}") for n in range(2)]
                for m in range(4)
            ]
            # Kt zero-padded per head-half: ktz[m][hl][c] is [128 outdim, 512 tokk]
            # where only partitions [hl*64, hl*64+64) hold data, the rest are 0.
            # Full-128-partition lhsT keeps the PE at 1 cyc/col (K=64 runs 2.5x
            # slower on real HW regardless of dtype).
            ktz = [
                [
                    [
                        kt_p.tile([128, 512], FP16, name=f"ktz{m}_{hl}_{c}")
                        for c in range(4)
                    ]
                    for hl in range(2)
                ]
                for m in range(4)
            ]
            for m in range(4):
                for c in range(4):
                    nc.vector.memset(ktz[m][0][c][64:128, :], 0.0)
                    nc.vector.memset(ktz[m][1][c][0:64, :], 0.0)
            # V_aug[t]: [128 tokk, 591]  (per-head 65-col blocks: 64 V dims,
            # col 64 = ones/denominator; cols 520+ are zeroed pad so head 7's
            # 128-col LDWEIGHTS window stays finite. PV loads a full 128-col
            # window per head — cols past 65 load the next head's data, which
            # only feeds ot partitions 65..127 that nobody reads — because a
            # 128-col stationary operand keeps FWL on (~25% faster matmuls).
            va_sb = [va_p.tile([128, GAP], FP16, name=f"va{t}") for t in range(16)]
            for t in range(16):
                nc.vector.memset(va_sb[t][:, GA:GAP], 0.0)
            # O^T (normalized) [concat dim 512 -> 4 tiles of 128, tokq 1024]
            osb = [osb_p.tile([128, NQ], MMDT, name=f"osb{t}") for t in range(4)]

            # fp16 stream buffers for qT/kT/vT.
            strm_tiles = [
                strm_p.tile([128, 512], FP16, name=f"strm{i}") for i in range(16)
            ]
            strm_ctr = [0]

            def strm():
                t = strm_tiles[strm_ctr[0] % 16]
                strm_ctr[0] += 1
                return t

            # ---- Phases B-D share the projection-weight scratch scope ----
            with (
                tc.tile_pool(name="wq_p", bufs=1) as wq_p,
                tc.tile_pool(name="wk_p", bufs=1) as wk_p,
                tc.tile_pool(name="wv_p", bufs=1) as wv_p,
            ):
                # DMA issue order: phase B's stream tiles interleaved with wq
                # first (each queue then carries only one ~128KB transfer
                # ahead of the first matmul; a queue moves ~20GB/s so every
                # weight tile queued ahead costs ~6us of PE start delay),
                # then wk/wv (trickle in during phase B), vbias/wo last
                # (needed at D-evac and F respectively).
                wq_sb = []
                qs_tiles = []
                for kk in range(8):
                    qs = strm()
                    nc.sync.dma_start(
                        qs[:], qT[kk * 128 : (kk + 1) * 128, 0:512]
                    )
                    t = wq_p.tile([128, GO], FP16, name=f"wq{kk}")
                    nc.sync.dma_start(t[:], wq[kk * 128 : (kk + 1) * 128, :])
                    wq_sb.append(t)
                    qs2 = strm()
                    nc.sync.dma_start(
                        qs2[:], qT[kk * 128 : (kk + 1) * 128, 512:1024]
                    )
                    qs_tiles.append((qs, qs2))
                wk_sb = []
                for kk in range(6):
                    t = wk_p.tile([128, GO], FP16, name=f"wk{kk}")
                    nc.sync.dma_start(t[:], wk[kk * 128 : (kk + 1) * 128, :])
                    wk_sb.append(t)
                wv_sb = []
                for kk in range(6):
                    t = wv_p.tile([128, GA], FP16, name=f"wv{kk}")
                    nc.sync.dma_start(t[:], wv[kk * 128 : (kk + 1) * 128, :])
                    wv_sb.append(t)

                # Phases B, C, D share ONE PSUM pool with 8 shared tile tags
                # (ps{i}_{j}, all [128,512] fp32, one bank each). A per-phase
                # pool scope would barrier the next phase's first matmul on
                # ALL eight evacuations + scope close (measured 5.3us at B->C
                # and 2.9us at C->D); shared tags instead make each bank's
                # next producer wait only that bank's own evacuation, which
                # lands while the other banks' matmuls still run.
                with tc.tile_pool(name="psP", bufs=1, space="PSUM") as psP:
                    # ---- Phase B: Q projection. Qt = Wq_g @ query^T (+bq) ----
                    psq = [
                        [
                            psP.tile([128, 512], FP32, name=f"ps{m}_{n}")
                            for n in range(2)
                        ]
                        for m in range(4)
                    ]
                    # n-outer so n=0's evacuation overlaps n=1's matmuls.
                    for n in range(2):
                        for kk in range(8):
                            qs = qs_tiles[kk][n]
                            for m in range(4):
                                _mm(
                                    nc,
                                    psq[m][n][:],
                                    wq_sb[kk][:, m * 128 : (m + 1) * 128],
                                    qs[:],
                                    start=(kk == 0),
                                    stop=(kk == 7),
                                )
                        for m in range(4):
                            if m % 2 == 0:
                                nc.vector.tensor_scalar_add(
                                    qt_sb[m][n][:], psq[m][n][:], bq_sb[:, m : m + 1]
                                )
                            else:
                                nc.scalar.activation(
                                    qt_sb[m][n][:],
                                    psq[m][n][:],
                                    mybir.ActivationFunctionType.Identity,
                                    bias=bq_sb[:, m : m + 1],
                                    scale=1.0,
                                )

                    # ---- Phase C: K projection. Kt = Wk_g @ key^T (+bk) ----
                    # (half, n)-outer: each 4-bank group's evacuation overlaps
                    # the next group's matmuls.
                    for half in range(2):
                        for n in range(2):
                            psk = [
                                psP.tile([128, 512], FP32, name=f"ps{m}_{n}")
                                for m in range(4)
                            ]
                            for kk in range(6):
                                ks_ = strm()
                                c0 = half * 1024 + n * 512
                                nc.sync.dma_start(
                                    ks_[:],
                                    kT[kk * 128 : (kk + 1) * 128, c0 : c0 + 512],
                                )
                                for m in range(4):
                                    _mm(
                                        nc,
                                        psk[m][:],
                                        wk_sb[kk][:, m * 128 : (m + 1) * 128],
                                        ks_[:],
                                        start=(kk == 0),
                                        stop=(kk == 5),
                                    )
                            for m in range(4):
                                c = half * 2 + n
                                nc.vector.tensor_scalar_add(
                                    ktz[m][0][c][0:64, :],
                                    psk[m][0:64, :],
                                    bk_sb[0:64, m : m + 1],
                                )
                                nc.scalar.activation(
                                    ktz[m][1][c][64:128, :],
                                    psk[m][64:128, :],
                                    mybir.ActivationFunctionType.Identity,
                                    bias=bk_sb[64:128, m : m + 1],
                                    scale=1.0,
                                )
                    # Heavy cold-phase loads issued after phase C's streams so
                    # no kT stream queues behind a 128KB wo half-tile: vbias
                    # is needed at D's first evacuation, wo only in phase F.
                    nc.sync.dma_start(vbias_sb[:, 0:HC], vbias[:, 0:HC])
                    nc.sync.dma_start(vbias_sb[:, HC:GA], vbias[:, HC:GA])
                    for kk in range(4):
                        for hv in range(2):
                            nc.sync.dma_start(
                                wo_sb[kk][:, hv * 512 : (hv + 1) * 512],
                                wo[
                                    kk * 128 : (kk + 1) * 128,
                                    hv * 512 : (hv + 1) * 512,
                                ],
                            )

                    # ---- Phase D: V_aug = value @ Wv_aug^T (+vbias) ----
                    # Packed 520-wide output (2 chunks of 260 per token tile).
                    for tb in range(4):
                        psv = [
                            [
                                psP.tile([128, 512], FP32, name=f"ps{t2}_{c}")
                                for c in range(2)
                            ]
                            for t2 in range(4)
                        ]
                        for kk in range(6):
                            vs = strm()
                            nc.sync.dma_start(
                                vs[:],
                                vT[kk * 128 : (kk + 1) * 128, tb * 512 : (tb + 1) * 512],
                            )
                            for t2 in range(4):
                                for c in range(2):
                                    _mm(
                                        nc,
                                        psv[t2][c][:, 0:HC],
                                        vs[:, t2 * 128 : (t2 + 1) * 128],
                                        wv_sb[kk][:, c * HC : (c + 1) * HC],
                                        start=(kk == 0),
                                        stop=(kk == 5),
                                    )
                        for t2 in range(4):
                            for c in range(2):
                                nc.vector.tensor_add(
                                    va_sb[tb * 4 + t2][:, c * HC : (c + 1) * HC],
                                    psv[t2][c][:, 0:HC],
                                    vbias_sb[:, c * HC : (c + 1) * HC],
                                )

            # ---- Phase E: attention per head (+ first half of out-proj) ----
            # Exp grouped into [128,1024] 2-bank ACT calls (halves per-call
            # overhead). Normalize chain (recip -> K=1 broadcast mm into the
            # pending ot's partitions 64..127 -> mul) is deferred by one
            # (h,n) iteration, and recip is enqueued on DVE BEFORE the flush
            # ops so the DVE FIFO can't hold it hostage behind a copy that
            # waits on the PE.
            # Iteration order is n-major: after the 8 (h, n=0) flushes the
            # osb columns 0..511 are final, so out-proj m-tiles 0..3 slot
            # into the PE slack of iterations 9..12 while ACT grinds exp.
            with (
                tc.tile_pool(name="otp", bufs=3, space="PSUM") as otp,
                tc.tile_pool(name="stp", bufs=2, space="PSUM") as stp,
                tc.tile_pool(name="psF", bufs=1, space="PSUM") as psF,
                tc.tile_pool(name="p_p", bufs=4) as p_p,
                tc.tile_pool(name="rc_p", bufs=2) as rc_p,
                tc.tile_pool(name="nr_p", bufs=2) as nr_p,
                tc.tile_pool(name="bcs_p", bufs=2) as bcs_p,
                tc.tile_pool(name="ys_p", bufs=2) as ys_p,
            ):
                iters = [(h, n) for n in range(2) for h in range(G)]
                # Two-deep normalize pipeline. A PE instruction's cross-engine
                # wait collapses onto the latest program-order-preceding
                # instruction of the depended-on engine, and that engine
                # completes in FIFO order — so the 3.4us single-partition DVE
                # InstReciprocal stalled every PE op with any DVE dependency
                # emitted in its shadow (~2us/iter, measured). Instead the
                # denominator row is reshaped onto 4 partitions by an
                # SBUF->SBUF DMA, the reciprocal runs partition-parallel as
                # [4,128] (~1.1us), a second DMA reshapes it back, and the
                # recip is emitted at pg==1 while the flush of the
                # two-iterations-ago head lands at pg==5 — so the only DVE
                # instructions next to PE emission points are sub-us copies.
                pending = [None, None]  # [one-ago, two-ago]
                stage1 = [None]  # entry awaiting its pg==1 reciprocal
                ot_cur = [None]

                def flush_oldest():
                    e = pending[1]
                    ot_p = e["ot"]
                    nc.tensor.matmul(
                        ot_p[64:128, :],
                        ones_sb[:, :],
                        e["rc"][:],
                        start=True,
                        stop=True,
                    )
                    bcs = bcs_p.tile([64, 512], FP16, name="bcs")
                    nc.vector.tensor_copy(bcs[:], ot_p[64:128, :])
                    nc.vector.tensor_mul(
                        osb[e["mt"]][
                            e["po"] : e["po"] + 64, e["n"] * 512 : (e["n"] + 1) * 512
                        ],
                        ot_p[:64, :],
                        bcs[:],
                    )
                    pending[1] = None

                def emit_recip():
                    e = stage1[0]
                    rc4 = nr_p.tile([4, 128], FP16, name="rc4")
                    nc.vector.reciprocal(rc4[:], e["den4"][:])
                    rc = rc_p.tile([1, 512], FP16, name="rc")
                    nc.sync.dma_start(rc[:], rc4[:])
                    e["rc"] = rc
                    stage1[0] = None

                def do_pv(pit, pg, pp):
                    ph, pn = iters[pit]
                    if pg == 0:
                        ot_cur[0] = otp.tile([128, 512], FP32, name="ot")
                    if pg == 1 and stage1[0] is not None:
                        emit_recip()
                    if pg == 5 and pending[1] is not None:
                        flush_oldest()
                    ot = ot_cur[0]
                    for j in range(2):
                        _mm(
                            nc,
                            ot[:],
                            va_sb[2 * pg + j][:, ph * DA : ph * DA + 128],
                            pp[:, j * 512 : (j + 1) * 512],
                            start=(pg == 0 and j == 0),
                            stop=(pg == 7 and j == 1),
                        )
                    if pg == 7:
                        # Denominator row off PSUM (sub-us DVE copy), then
                        # spread over 4 partitions for the cheap reciprocal.
                        den = nr_p.tile([1, 512], FP32, name="den")
                        nc.vector.tensor_copy(den[:], ot[64:65, :])
                        den4 = nr_p.tile([4, 128], FP32, name="den4")
                        nc.sync.dma_start(den4[:], den[:])
                        assert pending[1] is None
                        e = {
                            "ot": ot,
                            "den4": den4,
                            "rc": None,
                            "mt": ph // 2,
                            "po": (ph % 2) * 64,
                            "n": pn,
                        }
                        stage1[0] = e
                        pending[1] = pending[0]
                        pending[0] = e

                def do_F(m, psF_pool, ys_pool, dve_only):
                    ys = ys_pool.tile([128, 1024], FP32, name="ys")
                    for n2 in range(2):
                        psy = psF_pool.tile([128, 512], FP32, name="psy")
                        for kt in range(4):
                            _mm(
                                nc,
                                psy[:],
                                osb[kt][:, m * 128 : (m + 1) * 128],
                                wo_sb[kt][:, n2 * 512 : (n2 + 1) * 512],
                                start=(kt == 0),
                                stop=(kt == 3),
                            )
                        if dve_only or n2 == 0:
                            nc.vector.tensor_copy(
                                ys[:, n2 * 512 : (n2 + 1) * 512], psy[:]
                            )
                        else:
                            nc.scalar.copy(ys[:, 512:1024], psy[:])
                    for hv in range(2):
                        nc.sync.dma_start(
                            out[m * 128 : (m + 1) * 128, hv * 512 : (hv + 1) * 512],
                            ys[:, hv * 512 : (hv + 1) * 512],
                        )

                queue = []
                for it in range(16):
                    h, n = iters[it]
                    mt, hl = h // 2, h % 2
                    for g2 in range(8):
                        st2 = stp.tile([128, 1024], FP32, name="st")
                        for j in range(2):
                            kt = 2 * g2 + j
                            _mm(
                                nc,
                                st2[:, j * 512 : (j + 1) * 512],
                                ktz[mt][hl][kt // 4][
                                    :, (kt % 4) * 128 : (kt % 4 + 1) * 128
                                ],
                                qt_sb[mt][n][:],
                                start=True,
                                stop=True,
                            )
                        p2 = p_p.tile([128, 1024], FP16, name="p")
                        nc.scalar.activation(
                            p2[:],
                            st2[:],
                            mybir.ActivationFunctionType.Exp,
                            bias=0.0,
                            scale=SCALE,
                        )
                        queue.append((it, g2, p2))
                        if len(queue) >= 2:
                            do_pv(*queue.pop(0))
                    if 9 <= it <= 12:
                        # ACT is saturated with exp here; keep its queue clear.
                        do_F(it - 9, psF, ys_p, dve_only=True)
                do_pv(*queue.pop(0))
                if stage1[0] is not None:
                    emit_recip()
                if pending[1] is not None:
                    flush_oldest()
                pending[1] = pending[0]
                pending[0] = None
                flush_oldest()

            # ---- Phase F tail: out-proj m-tiles 4..7 (tokq 512..1023) ----
            with (
                tc.tile_pool(name="psF2", bufs=4, space="PSUM") as psF2,
                tc.tile_pool(name="ys2_p", bufs=2) as ys2_p,
            ):
                for m in range(4, 8):
                    do_F(m, psF2, ys2_p, dve_only=False)

    nc.finalize()
    return nc


def kernel(**inputs):
    global LAST_RESULT
    arrs = {k: np.asarray(v, dtype=np.float32) for k, v in inputs.items()}
    query, key, value = arrs["query"], arrs["key"], arrs["value"]
    Wq, bq_, Wk, bk_ = arrs["Wq"], arrs["bq"], arrs["Wk"], arrs["bk"]
    Wv, bv_, Wo, bo_ = arrs["Wv"], arrs["bv"], arrs["Wo"], arrs["bo"]

    nc = build_program()

    qTb = [np.ascontiguousarray(query[b].T.astype(np.float16)) for b in range(B)]
    kTb = [np.ascontiguousarray(key[b].T.astype(np.float16)) for b in range(B)]
    vTb = [np.ascontiguousarray(value[b].T.astype(np.float16)) for b in range(B)]

    per_group = []
    for g in range(2):
        gs = slice(g * GO, (g + 1) * GO)
        wq_m = np.ascontiguousarray(Wq[gs, :].T.astype(np.float16))
        wk_m = np.ascontiguousarray(Wk[gs, :].T.astype(np.float16))
        wv_aug = np.zeros((KD, GA), np.float32)
        vb_row = np.zeros((GA,), np.float32)
        for h in range(G):
            hs = slice(g * GO + h * D, g * GO + (h + 1) * D)
            wv_aug[:, h * DA : h * DA + D] = Wv[hs, :].T
            vb_row[h * DA : h * DA + D] = bv_[hs]
            vb_row[h * DA + D] = 1.0
        vbias_m = np.ascontiguousarray(np.tile(vb_row, (128, 1)).astype(np.float32))
        wo_m = np.ascontiguousarray(Wo[:, gs].T.astype(np.float16))
        bq_m = np.ascontiguousarray(bq_[gs].reshape(4, 128).T)
        bk_m = np.ascontiguousarray(bk_[gs].reshape(4, 128).T)
        per_group.append(
            {
                "wq": wq_m,
                "wk": wk_m,
                "wv": wv_aug.astype(np.float16),
                "wo": wo_m,
                "vbias": vbias_m,
                "bq": bq_m,
                "bk": bk_m,
            }
        )

    in_maps = []
    for c in range(8):
        b, g = c // 2, c % 2
        m = {"qT": qTb[b], "kT": kTb[b], "vT": vTb[b]}
        m.update(per_group[g])
        m["rev11"] = np.zeros((1, 4), np.float32)
        in_maps.append(m)

    res = run_bass_kernel_spmd(
        nc, in_maps, list(range(8)), trace=TRACE, **(TRACE_KWARGS if TRACE else {})
    )
    LAST_RESULT = res

    outs = res.results
    Y = np.empty((B, NQ, E), np.float32)
    for b in range(B):
        Y[b] = outs[2 * b]["out"] + outs[2 * b + 1]["out"] + bo_[None, :]
    return Y


# revision 50
# speedup vs baseline: 1.0218x; 1.0085x over previous
"""MultiHeadCrossAttention on 8 TRN2 NeuronCores.

Sharding: core c -> batch b = c//2, head-group g = c%2 (8 heads, 512 out dims).
Each core computes its head-group's Q/K/V projections, attention, and a
partial out-projection (Wo columns restricted to its head-group). Host sums
the two partials per batch and adds bo.

Softmax denominator trick: V is augmented with a ones-column per head
(via a zero weight column + bias 1.0), so attn@V_aug row 64 of each head's
65-wide block is the softmax denominator; normalize with DVE reciprocal +
a K=1 matmul broadcast, deferred one iteration so the reciprocal latency
hides under the next head's compute.

V_aug is packed 65 cols/head (520 total, not 128-padded): the V projection
streams only 520 output columns (PE cost is N-proportional), and PV runs
M=65 which costs the same as M=128.

PSUM evacuations alternate between DVE and ACT so phase-boundary drains
(all 8 banks at once) take half the serial time; ACT is otherwise idle
outside the exp phase.

All stream DMAs rotate through one shared pool with bufs == NUM_HWDGE_SEMS
(8) so slot reuse lands on the same DMA lane (WAW collapses into queue
order) and every DMA carries at most 2 semaphore waits (walrus limit).
"""

import sys

import numpy as np

if "/opt/trn_rl_repo" not in sys.path:
    sys.path.insert(0, "/opt/trn_rl_repo")

import concourse.bacc as bacc
import concourse.bass as bass
import concourse.mybir as mybir
import concourse.tile as tile
from concourse.bass_utils import run_bass_kernel_spmd

FP32 = mybir.dt.float32
FP32R = mybir.dt.float32r
FP16 = mybir.dt.float16

B, NQ, NK = 4, 1024, 2048
QD, KD = 1024, 768
H, D = 16, 64
E = H * D  # 1024 total embed dim
G = 8  # heads per core
GO = G * D  # 512 out dims per core
DA = D + 1  # 65: head dim + denominator column
GA = G * DA  # 520
HC = GA // 2  # 260: half of the packed V_aug width
GAP = 7 * DA + 128 + 8  # 591: V_aug SBUF width so every head has a 128-col LDW window
SCALE = 1.0 / 8.0

MMDT = FP16

# test.py hooks
TRACE = False
TRACE_KWARGS = {}
LAST_RESULT = None


def _mm(nc, out, lhsT, rhs, start, stop):
    nc.tensor.matmul(out, lhsT, rhs, start=start, stop=stop)


def build_program():
    nc = bacc.Bacc()

    qT = nc.declare_dram_parameter("qT", [QD, NQ], FP16, isOutput=False)
    kT = nc.declare_dram_parameter("kT", [KD, NK], FP16, isOutput=False)
    vT = nc.declare_dram_parameter("vT", [KD, NK], FP16, isOutput=False)
    wq = nc.declare_dram_parameter("wq", [QD, GO], FP16, isOutput=False)
    wk = nc.declare_dram_parameter("wk", [KD, GO], FP16, isOutput=False)
    wv = nc.declare_dram_parameter("wv", [KD, GA], FP16, isOutput=False)
    wo = nc.declare_dram_parameter("wo", [GO, E], MMDT, isOutput=False)
    vbias = nc.declare_dram_parameter("vbias", [128, GA], FP32, isOutput=False)
    bq = nc.declare_dram_parameter("bq", [128, 4], FP32, isOutput=False)
    bk = nc.declare_dram_parameter("bk", [128, 4], FP32, isOutput=False)
    out = nc.declare_dram_parameter("out", [NQ, E], FP32, isOutput=True)

    with (
        nc.allow_low_precision("fp16 attention activations; validated 1.7e-4 rel"),
        tile.TileContext(nc) as tc,
    ):
        with (
            tc.tile_pool(name="consts", bufs=1) as consts,
            tc.tile_pool(name="wo_p", bufs=1) as wo_p,
            tc.tile_pool(name="qt_p", bufs=1) as qt_p,
            tc.tile_pool(name="kt_p", bufs=1) as kt_p,
            tc.tile_pool(name="va_p", bufs=1) as va_p,
            tc.tile_pool(name="osb_p", bufs=1) as osb_p,
            tc.tile_pool(name="strm_p", bufs=1) as strm_p,
        ):
            bq_sb = consts.tile([128, 4], FP32)
            nc.sync.dma_start(bq_sb[:], bq[:, :])
            bk_sb = consts.tile([128, 4], FP32)
            nc.sync.dma_start(bk_sb[:], bk[:, :])
            ones_sb = consts.tile([1, 64], FP16)
            nc.vector.memset(ones_sb[:], 1.0)
            vbias_sb = consts.tile([128, GA], FP32)
            wo_sb = [wo_p.tile([128, E], MMDT, name=f"wo{kk}") for kk in range(4)]

            # Persistent activation tiles.
            # Qt[m][n]: [128 outdim, 512 tokq]   (m: outdim tile, n: tokq chunk)
            qt_sb = [
                [qt_p.tile([128, 512], FP16, name=f"qt{m}_{n}") for n in range(2)]
                for m in range(4)
            ]
            # Kt zero-padded per head-half: ktz[m][hl][c] is [128 outdim, 512 tokk]
            # where only partitions [hl*64, hl*64+64) hold data, the rest are 0.
            # Full-128-partition lhsT keeps the PE at 1 cyc/col (K=64 runs 2.5x
            # slower on real HW regardless of dtype).
            ktz = [
                [
                    [
                        kt_p.tile([128, 512], FP16, name=f"ktz{m}_{hl}_{c}")
                        for c in range(4)
                    ]
                    for hl in range(2)
                ]
                for m in range(4)
            ]
            for m in range(4):
                for c in range(4):
                    nc.vector.memset(ktz[m][0][c][64:128, :], 0.0)
                    nc.vector.memset(ktz[m][1][c][0:64, :], 0.0)
            # V_aug[t]: [128 tokk, 591]  (per-head 65-col blocks: 64 V dims,
            # col 64 = ones/denominator; cols 520+ are zeroed pad so head 7's
            # 128-col LDWEIGHTS window stays finite. PV loads a full 128-col
            # window per head — cols past 65 load the next head's data, which
            # only feeds ot partitions 65..127 that nobody reads — because a
            # 128-col stationary operand keeps FWL on (~25% faster matmuls).
            va_sb = [va_p.tile([128, GAP], FP16, name=f"va{t}") for t in range(16)]
            for t in range(16):
                nc.vector.memset(va_sb[t][:, GA:GAP], 0.0)
            # O^T (normalized) [concat dim 512 -> 4 tiles of 128, tokq 1024]
            osb = [osb_p.tile([128, NQ], MMDT, name=f"osb{t}") for t in range(4)]

            # fp16 stream buffers for qT/kT/vT.
            strm_tiles = [
                strm_p.tile([128, 512], FP16, name=f"strm{i}") for i in range(16)
            ]
            strm_ctr = [0]

            def strm():
                t = strm_tiles[strm_ctr[0] % 16]
                strm_ctr[0] += 1
                return t

            # ---- Phases B-D share the projection-weight scratch scope ----
            with (
                tc.tile_pool(name="wq_p", bufs=1) as wq_p,
                tc.tile_pool(name="wk_p", bufs=1) as wk_p,
                tc.tile_pool(name="wv_p", bufs=1) as wv_p,
            ):
                # DMA issue order: phase B's stream tiles interleaved with wq
                # first (each queue then carries only one ~128KB transfer
                # ahead of the first matmul; a queue moves ~20GB/s so every
                # weight tile queued ahead costs ~6us of PE start delay),
                # then wk/wv (trickle in during phase B), vbias/wo last
                # (needed at D-evac and F respectively).
                wq_sb = []
                qs_tiles = []
                for kk in range(8):
                    qs = strm()
                    nc.sync.dma_start(
                        qs[:], qT[kk * 128 : (kk + 1) * 128, 0:512]
                    )
                    t = wq_p.tile([128, GO], FP16, name=f"wq{kk}")
                    nc.sync.dma_start(t[:], wq[kk * 128 : (kk + 1) * 128, :])
                    wq_sb.append(t)
                    qs2 = strm()
                    nc.sync.dma_start(
                        qs2[:], qT[kk * 128 : (kk + 1) * 128, 512:1024]
                    )
                    qs_tiles.append((qs, qs2))
                wk_sb = []
                for kk in range(6):
                    t = wk_p.tile([128, GO], FP16, name=f"wk{kk}")
                    nc.sync.dma_start(t[:], wk[kk * 128 : (kk + 1) * 128, :])
                    wk_sb.append(t)
                wv_sb = []
                for kk in range(6):
                    t = wv_p.tile([128, GA], FP16, name=f"wv{kk}")
                    nc.sync.dma_start(t[:], wv[kk * 128 : (kk + 1) * 128, :])
                    wv_sb.append(t)

                # ---- Phase B: Q projection. Qt = Wq_g @ query^T (+bq) ----
                with tc.tile_pool(name="psB", bufs=1, space="PSUM") as psB:
                    psq = [
                        [
                            psB.tile([128, 512], FP32, name=f"psq{m}_{n}")
                            for n in range(2)
                        ]
                        for m in range(4)
                    ]
                    # n-outer so n=0's evacuation overlaps n=1's matmuls.
                    for n in range(2):
                        for kk in range(8):
                            qs = qs_tiles[kk][n]
                            for m in range(4):
                                _mm(
                                    nc,
                                    psq[m][n][:],
                                    wq_sb[kk][:, m * 128 : (m + 1) * 128],
                                    qs[:],
                                    start=(kk == 0),
                                    stop=(kk == 7),
                                )
                        for m in range(4):
                            if m % 2 == 0:
                                nc.vector.tensor_scalar_add(
                                    qt_sb[m][n][:], psq[m][n][:], bq_sb[:, m : m + 1]
                                )
                            else:
                                nc.scalar.activation(
                                    qt_sb[m][n][:],
                                    psq[m][n][:],
                                    mybir.ActivationFunctionType.Identity,
                                    bias=bq_sb[:, m : m + 1],
                                    scale=1.0,
                                )

                # ---- Phase C: K projection. Kt = Wk_g @ key^T (+bk) ----
                # (half, n)-outer: each 4-bank group's evacuation overlaps the
                # next group's matmuls.
                with tc.tile_pool(name="psC", bufs=1, space="PSUM") as psC:
                    for half in range(2):
                        for n in range(2):
                            psk = [
                                psC.tile([128, 512], FP32, name=f"psk{m}_{n}")
                                for m in range(4)
                            ]
                            for kk in range(6):
                                ks_ = strm()
                                c0 = half * 1024 + n * 512
                                nc.sync.dma_start(
                                    ks_[:],
                                    kT[kk * 128 : (kk + 1) * 128, c0 : c0 + 512],
                                )
                                for m in range(4):
                                    _mm(
                                        nc,
                                        psk[m][:],
                                        wk_sb[kk][:, m * 128 : (m + 1) * 128],
                                        ks_[:],
                                        start=(kk == 0),
                                        stop=(kk == 5),
                                    )
                            for m in range(4):
                                c = half * 2 + n
                                nc.vector.tensor_scalar_add(
                                    ktz[m][0][c][0:64, :],
                                    psk[m][0:64, :],
                                    bk_sb[0:64, m : m + 1],
                                )
                                nc.scalar.activation(
                                    ktz[m][1][c][64:128, :],
                                    psk[m][64:128, :],
                                    mybir.ActivationFunctionType.Identity,
                                    bias=bk_sb[64:128, m : m + 1],
                                    scale=1.0,
                                )
                # Heavy cold-phase loads issued after phase C's streams so no
                # kT stream queues behind a 128KB wo half-tile: vbias is
                # needed at D's first evacuation, wo only in phase F.
                nc.sync.dma_start(vbias_sb[:, 0:HC], vbias[:, 0:HC])
                nc.sync.dma_start(vbias_sb[:, HC:GA], vbias[:, HC:GA])
                for kk in range(4):
                    for hv in range(2):
                        nc.sync.dma_start(
                            wo_sb[kk][:, hv * 512 : (hv + 1) * 512],
                            wo[kk * 128 : (kk + 1) * 128, hv * 512 : (hv + 1) * 512],
                        )

                # ---- Phase D: V_aug = value @ Wv_aug^T (+vbias, ones col) ----
                # Packed 520-wide output (2 chunks of 260 per token tile).
                with tc.tile_pool(name="psD", bufs=1, space="PSUM") as psD:
                    for tb in range(4):
                        psv = [
                            [
                                psD.tile([128, 512], FP32, name=f"psv{t2}_{c}")
                                for c in range(2)
                            ]
                            for t2 in range(4)
                        ]
                        for kk in range(6):
                            vs = strm()
                            nc.sync.dma_start(
                                vs[:],
                                vT[kk * 128 : (kk + 1) * 128, tb * 512 : (tb + 1) * 512],
                            )
                            for t2 in range(4):
                                for c in range(2):
                                    _mm(
                                        nc,
                                        psv[t2][c][:, 0:HC],
                                        vs[:, t2 * 128 : (t2 + 1) * 128],
                                        wv_sb[kk][:, c * HC : (c + 1) * HC],
                                        start=(kk == 0),
                                        stop=(kk == 5),
                                    )
                        for t2 in range(4):
                            for c in range(2):
                                nc.vector.tensor_add(
                                    va_sb[tb * 4 + t2][:, c * HC : (c + 1) * HC],
                                    psv[t2][c][:, 0:HC],
                                    vbias_sb[:, c * HC : (c + 1) * HC],
                                )

            # ---- Phase E: attention per head (+ first half of out-proj) ----
            # Exp grouped into [128,1024] 2-bank ACT calls (halves per-call
            # overhead). Normalize chain (recip -> K=1 broadcast mm into the
            # pending ot's partitions 64..127 -> mul) is deferred by one
            # (h,n) iteration, and recip is enqueued on DVE BEFORE the flush
            # ops so the DVE FIFO can't hold it hostage behind a copy that
            # waits on the PE.
            # Iteration order is n-major: after the 8 (h, n=0) flushes the
            # osb columns 0..511 are final, so out-proj m-tiles 0..3 slot
            # into the PE slack of iterations 9..12 while ACT grinds exp.
            with (
                tc.tile_pool(name="otp", bufs=3, space="PSUM") as otp,
                tc.tile_pool(name="stp", bufs=2, space="PSUM") as stp,
                tc.tile_pool(name="psF", bufs=1, space="PSUM") as psF,
                tc.tile_pool(name="p_p", bufs=4) as p_p,
                tc.tile_pool(name="rc_p", bufs=2) as rc_p,
                tc.tile_pool(name="nr_p", bufs=2) as nr_p,
                tc.tile_pool(name="bcs_p", bufs=2) as bcs_p,
                tc.tile_pool(name="ys_p", bufs=2) as ys_p,
            ):
                iters = [(h, n) for n in range(2) for h in range(G)]
                # Two-deep normalize pipeline. A PE instruction's cross-engine
                # wait collapses onto the latest program-order-preceding
                # instruction of the depended-on engine, and that engine
                # completes in FIFO order — so the 3.4us single-partition DVE
                # InstReciprocal stalled every PE op with any DVE dependency
                # emitted in its shadow (~2us/iter, measured). Instead the
                # denominator row is reshaped onto 4 partitions by an
                # SBUF->SBUF DMA, the reciprocal runs partition-parallel as
                # [4,128] (~1.1us), a second DMA reshapes it back, and the
                # recip is emitted at pg==1 while the flush of the
                # two-iterations-ago head lands at pg==5 — so the only DVE
                # instructions next to PE emission points are sub-us copies.
                pending = [None, None]  # [one-ago, two-ago]
                stage1 = [None]  # entry awaiting its pg==1 reciprocal
                ot_cur = [None]

                def flush_oldest():
                    e = pending[1]
                    ot_p = e["ot"]
                    nc.tensor.matmul(
                        ot_p[64:128, :],
                        ones_sb[:, :],
                        e["rc"][:],
                        start=True,
                        stop=True,
                    )
                    bcs = bcs_p.tile([64, 512], FP16, name="bcs")
                    nc.vector.tensor_copy(bcs[:], ot_p[64:128, :])
                    nc.vector.tensor_mul(
                        osb[e["mt"]][
                            e["po"] : e["po"] + 64, e["n"] * 512 : (e["n"] + 1) * 512
                        ],
                        ot_p[:64, :],
                        bcs[:],
                    )
                    pending[1] = None

                def emit_recip():
                    e = stage1[0]
                    rc4 = nr_p.tile([4, 128], FP16, name="rc4")
                    nc.vector.reciprocal(rc4[:], e["den4"][:])
                    rc = rc_p.tile([1, 512], FP16, name="rc")
                    nc.sync.dma_start(rc[:], rc4[:])
                    e["rc"] = rc
                    stage1[0] = None

                def do_pv(pit, pg, pp):
                    ph, pn = iters[pit]
                    if pg == 0:
                        ot_cur[0] = otp.tile([128, 512], FP32, name="ot")
                    if pg == 1 and stage1[0] is not None:
                        emit_recip()
                    if pg == 5 and pending[1] is not None:
                        flush_oldest()
                    ot = ot_cur[0]
                    for j in range(2):
                        _mm(
                            nc,
                            ot[:],
                            va_sb[2 * pg + j][:, ph * DA : ph * DA + 128],
                            pp[:, j * 512 : (j + 1) * 512],
                            start=(pg == 0 and j == 0),
                            stop=(pg == 7 and j == 1),
                        )
                    if pg == 7:
                        # Denominator row off PSUM (sub-us DVE copy), then
                        # spread over 4 partitions for the cheap reciprocal.
                        den = nr_p.tile([1, 512], FP32, name="den")
                        nc.vector.tensor_copy(den[:], ot[64:65, :])
                        den4 = nr_p.tile([4, 128], FP32, name="den4")
                        nc.sync.dma_start(den4[:], den[:])
                        assert pending[1] is None
                        e = {
                            "ot": ot,
                            "den4": den4,
                            "rc": None,
                            "mt": ph // 2,
                            "po": (ph % 2) * 64,
                            "n": pn,
                        }
                        stage1[0] = e
                        pending[1] = pending[0]
                        pending[0] = e

                def do_F(m, psF_pool, ys_pool, dve_only):
                    ys = ys_pool.tile([128, 1024], FP32, name="ys")
                    for n2 in range(2):
                        psy = psF_pool.tile([128, 512], FP32, name="psy")
                        for kt in range(4):
                            _mm(
                                nc,
                                psy[:],
                                osb[kt][:, m * 128 : (m + 1) * 128],
                                wo_sb[kt][:, n2 * 512 : (n2 + 1) * 512],
                                start=(kt == 0),
                                stop=(kt == 3),
                            )
                        if dve_only or n2 == 0:
                            nc.vector.tensor_copy(
                                ys[:, n2 * 512 : (n2 + 1) * 512], psy[:]
                            )
                        else:
                            nc.scalar.copy(ys[:, 512:1024], psy[:])
                    for hv in range(2):
                        nc.sync.dma_start(
                            out[m * 128 : (m + 1) * 128, hv * 512 : (hv + 1) * 512],
                            ys[:, hv * 512 : (hv + 1) * 512],
                        )

                queue = []
                for it in range(16):
                    h, n = iters[it]
                    mt, hl = h // 2, h % 2
                    for g2 in range(8):
                        st2 = stp.tile([128, 1024], FP32, name="st")
                        for j in range(2):
                            kt = 2 * g2 + j
                            _mm(
                                nc,
                                st2[:, j * 512 : (j + 1) * 512],
                                ktz[mt][hl][kt // 4][
                                    :, (kt % 4) * 128 : (kt % 4 + 1) * 128
                                ],
                                qt_sb[mt][n][:],
                                start=True,
                                stop=True,
                            )
                        p2 = p_p.tile([128, 1024], FP16, name="p")
                        nc.scalar.activation(
                            p2[:],
                            st2[:],
                            mybir.ActivationFunctionType.Exp,
                            bias=0.0,
                            scale=SCALE,
                        )
                        queue.append((it, g2, p2))
                        if len(queue) >= 2:
                            do_pv(*queue.pop(0))
                    if 9 <= it <= 12:
                        # ACT is saturated with exp here; keep its queue clear.
                        do_F(it - 9, psF, ys_p, dve_only=True)
                do_pv(*queue.pop(0))
                if stage1[0] is not None:
                    emit_recip()
                if pending[1] is not None:
                    flush_oldest()
                pending[1] = pending[0]
                pending[0] = None
                flush_oldest()

            # ---- Phase F tail: out-proj m-tiles 4..7 (tokq 512..1023) ----
            with (
                tc.tile_pool(name="psF2", bufs=4, space="PSUM") as psF2,
                tc.tile_pool(name="ys2_p", bufs=2) as ys2_p,
            ):
                for m in range(4, 8):
                    do_F(m, psF2, ys2_p, dve_only=False)

    nc.finalize()
    return nc


def kernel(**inputs):
    global LAST_RESULT
    arrs = {k: np.asarray(v, dtype=np.float32) for k, v in inputs.items()}
    query, key, value = arrs["query"], arrs["key"], arrs["value"]
    Wq, bq_, Wk, bk_ = arrs["Wq"], arrs["bq"], arrs["Wk"], arrs["bk"]
    Wv, bv_, Wo, bo_ = arrs["Wv"], arrs["bv"], arrs["Wo"], arrs["bo"]

    nc = build_program()

    qTb = [np.ascontiguousarray(query[b].T.astype(np.float16)) for b in range(B)]
    kTb = [np.ascontiguousarray(key[b].T.astype(np.float16)) for b in range(B)]
    vTb = [np.ascontiguousarray(value[b].T.astype(np.float16)) for b in range(B)]

    per_group = []
    for g in range(2):
        gs = slice(g * GO, (g + 1) * GO)
        wq_m = np.ascontiguousarray(Wq[gs, :].T.astype(np.float16))
        wk_m = np.ascontiguousarray(Wk[gs, :].T.astype(np.float16))
        wv_aug = np.zeros((KD, GA), np.float32)
        vb_row = np.zeros((GA,), np.float32)
        for h in range(G):
            hs = slice(g * GO + h * D, g * GO + (h + 1) * D)
            wv_aug[:, h * DA : h * DA + D] = Wv[hs, :].T
            vb_row[h * DA : h * DA + D] = bv_[hs]
            vb_row[h * DA + D] = 1.0
        vbias_m = np.ascontiguousarray(np.tile(vb_row, (128, 1)).astype(np.float32))
        wo_m = np.ascontiguousarray(Wo[:, gs].T.astype(np.float16))
        bq_m = np.ascontiguousarray(bq_[gs].reshape(4, 128).T)
        bk_m = np.ascontiguousarray(bk_[gs].reshape(4, 128).T)
        per_group.append(
            {
                "wq": wq_m,
                "wk": wk_m,
                "wv": wv_aug.astype(np.float16),
                "wo": wo_m,
                "vbias": vbias_m,
                "bq": bq_m,
                "bk": bk_m,
            }
        )

    in_maps = []
    for c in range(8):
        b, g = c // 2, c % 2
        m = {"qT": qTb[b], "kT": kTb[b], "vT": vTb[b]}
        m.update(per_group[g])
        in_maps.append(m)

    res = run_bass_kernel_spmd(
        nc, in_maps, list(range(8)), trace=TRACE, **(TRACE_KWARGS if TRACE else {})
    )
    LAST_RESULT = res

    outs = res.results
    Y = np.empty((B, NQ, E), np.float32)
    for b in range(B):
        Y[b] = outs[2 * b]["out"] + outs[2 * b + 1]["out"] + bo_[None, :]
    return Y
